# revision 23
# baseline (speedup 1.0000x reference)
"""HOI relation-scoring kernel for Trainium2 (8 NeuronCores, data-parallel).

Full inputs in, full output out. Batch dim (16 images) sharded 2-per-core
across 8 cores; MLP weights replicated.

Per-core dataflow (2 images):
  - Features host-repacked to [128, 43, 768] bf16 per image (flat yx =
    43*p + c, padded 5476 -> 5504): per-partition-contiguous DMA lines,
    no tail chunk; pad rows neutralized via coordinate constants (-1e9).
  - Weights host-permuted to the SBUF chunk layout [p, c, m]: descriptor
    generation is 128 contiguous lines instead of 1536 strided ones
    (the rearranged form cost ~9 us of ACT-sequencer DIRECT2D time and
    stalled everything behind it).
  - Box DMAs ride the SP ring AHEAD of the feature groups; the whole
    box -> mask pipeline runs on the Vector engine in bf16 (2x DVE) and
    is built in two column pieces so the first ROI matmuls start as soon
    as feature group 0 lands. Both images' masks are built before any
    ROI-consuming DVE work so image 1's ROI is never mask-gated.
  - ROI sums: mask chunk [128, 24] bf16 stationary, feature chunk
    [128, 768] bf16 streams -> psum [24, 512] + [24, 256], 43 chunks.
  - MLP in float32r (f32 matmuls lower to 2 half-speed PE instructions;
    f32r does not). Stage 1 is ROI-stationary (lhsT = roiT columns,
    rhs = W1 row-chunk [128, 512]) and runs per image so image 0's half
    fills PE wait gaps during image 1's ROI. Pair expansion
    h1T[d1, (img,i,j)] = A.T + b1 + B.T is a scalar_tensor_tensor per
    d1 chunk on the DVE; stages 2/3 chain in transposed layout
    (weight-stationary, N=256 / N=128 streams, W3 host-padded to 128).
"""

import sys
import types

import numpy as np

for _p in ("/opt/trn_rl_repo",):
    if _p not in sys.path:
        sys.path.insert(0, _p)

from contextlib import ExitStack

from concourse import bacc, mybir, tile
from concourse.bass import ts
from concourse.bass_utils import run_bass_kernel_spmd
from concourse.masks import make_identity

# Problem shapes (hardcoded per contract).
B, H, W, D = 16, 74, 74, 768
NH, NO = 8, 16
NB = NH + NO  # 24 boxes per image
NREL = 117
D1, D2 = 512, 256
NCORES = 8
BPC = B // NCORES  # images per core
YX = H * W  # 5476
NCHUNK = 43
PADYX = 128 * NCHUNK  # 5504
NPAIR = NH * NO  # 128 pairs per image
GROUPS = [(0, 3), (3, 8), (8, 14), (14, 20), (20, 26), (26, 32), (32, 38), (38, 43)]
GMAX = max(c1 - c0 for c0, c1 in GROUPS)
MASK_PIECES = [(0, 3), (3, 43)]

F32 = mybir.dt.float32
FR = mybir.dt.float32r
BF16 = mybir.dt.bfloat16
I32 = mybir.dt.int32

K1 = 2 * D // 128  # 12 chunks of W1 rows (first 6 = human half)
K2 = D1 // 128  # 4
K3 = D2 // 128  # 2
MC1 = D1 // 128  # 4
MC2 = D2 // 128  # 2
DCH = D // 128  # 6

_CACHE = {}


def _coord_consts():
    """Host constants: partition/chunk -> (y, x) coordinate grids [128, 43].

    flat = 43*p + c; invalid (pad) entries get -1e9 so all box compares
    fail and pad rows contribute zero to the ROI sums. bf16 (coords are
    small integers, exact; enables 2x DVE compares).
    """
    import ml_dtypes

    p = np.arange(128)[:, None]
    c = np.arange(NCHUNK)[None, :]
    flat = NCHUNK * p + c
    valid = flat < YX
    y = np.where(valid, flat // W, -1e9).astype(np.float32)
    x = np.where(valid, flat % W, -1e9).astype(np.float32)
    return (
        np.ascontiguousarray(y.astype(ml_dtypes.bfloat16)),
        np.ascontiguousarray(x.astype(ml_dtypes.bfloat16)),
    )


def _mask_piece(nc, mask, mwrk, mtmp, ycosb, xcosb, bcbf, c0, c1):
    """Columns [c0, c1) of one image's indicator mask, all-bf16 on DVE."""
    w = c1 - c0
    shp = (128, w, NB)
    yv = ycosb[:, c0:c1].unsqueeze(2).to_broadcast(shp)
    xv = xcosb[:, c0:c1].unsqueeze(2).to_broadcast(shp)
    x1v = bcbf[:, 0:NB].unsqueeze(1).to_broadcast(shp)
    y1v = bcbf[:, NB : 2 * NB].unsqueeze(1).to_broadcast(shp)
    x2v = bcbf[:, 2 * NB : 3 * NB].unsqueeze(1).to_broadcast(shp)
    y2v = bcbf[:, 3 * NB : 4 * NB].unsqueeze(1).to_broadcast(shp)
    mw = mwrk[:, c0:c1, :]
    mt = mtmp[:, c0:c1, :]
    nc.vector.tensor_tensor(mw, yv, y1v, mybir.AluOpType.is_ge)
    nc.vector.tensor_tensor(mt, yv, y2v, mybir.AluOpType.is_lt)
    nc.vector.tensor_mul(mw, mw, mt)
    nc.vector.tensor_tensor(mt, xv, x1v, mybir.AluOpType.is_ge)
    nc.vector.tensor_mul(mw, mw, mt)
    nc.vector.tensor_tensor(mt, xv, x2v, mybir.AluOpType.is_lt)
    nc.vector.tensor_mul(mask[:, c0:c1, :], mw, mt)


def _build_nc():
    nc = bacc.Bacc("TRN2", target_bir_lowering=False)

    feats = nc.dram_tensor("feats", [BPC, 128, NCHUNK, D], BF16, kind="ExternalInput")
    hbox = nc.dram_tensor("hbox", [BPC, NH, 4], I32, kind="ExternalInput")
    obox = nc.dram_tensor("obox", [BPC, NO, 4], I32, kind="ExternalInput")
    # weights host-permuted to [p, chunk, m]; W3 host-padded to 128 cols
    w1 = nc.dram_tensor("w1", [128, K1, D1], FR, kind="ExternalInput")
    b1 = nc.dram_tensor("b1", [D1], F32, kind="ExternalInput")
    w2 = nc.dram_tensor("w2", [128, K2, D2], FR, kind="ExternalInput")
    b2 = nc.dram_tensor("b2", [D2], F32, kind="ExternalInput")
    w3 = nc.dram_tensor("w3", [128, K3, 128], FR, kind="ExternalInput")
    b3 = nc.dram_tensor("b3", [NREL], F32, kind="ExternalInput")
    yco = nc.dram_tensor("yco", [128, NCHUNK], BF16, kind="ExternalInput")
    xco = nc.dram_tensor("xco", [128, NCHUNK], BF16, kind="ExternalInput")
    out = nc.dram_tensor("out", [BPC * NPAIR, NREL], F32, kind="ExternalOutput")

    with tile.TileContext(nc) as tc, ExitStack() as ctx:
        const = ctx.enter_context(tc.tile_pool(name="const", bufs=1))
        fpool = ctx.enter_context(tc.tile_pool(name="fpool", bufs=8))
        wpool = ctx.enter_context(tc.tile_pool(name="wpool", bufs=1))
        spool = ctx.enter_context(tc.tile_pool(name="spool", bufs=2))
        roi_ps = ctx.enter_context(tc.tile_pool(name="roi_ps", bufs=2, space="PSUM"))
        ab_ps = ctx.enter_context(tc.tile_pool(name="ab_ps", bufs=1, space="PSUM"))
        sm_ps = ctx.enter_context(tc.tile_pool(name="sm_ps", bufs=2, space="PSUM"))

        # ---- ACT ring: small constants go first (cheap descriptor gen)
        ycosb = const.tile([128, NCHUNK], BF16, tag="ycosb")
        nc.scalar.dma_start(ycosb[:], yco[:])
        xcosb = const.tile([128, NCHUNK], BF16, tag="xcosb")
        nc.scalar.dma_start(xcosb[:], xco[:])
        b1sb = const.tile([128, MC1], F32, tag="b1")
        nc.scalar.dma_start(b1sb[:], b1[:].rearrange("(c p) -> p c", p=128))
        b2sb = const.tile([128, MC2], F32, tag="b2")
        nc.scalar.dma_start(b2sb[:], b2[:].rearrange("(c p) -> p c", p=128))
        b3bc = const.tile([128, NREL], F32, tag="b3bc")
        nc.scalar.dma_start(b3bc[:], b3[None, :].to_broadcast((128, NREL)))

        # ---- SP ring: tiny box DMAs first, then the feature stream
        boxis = []
        for b in range(BPC):
            boxi = const.tile([1, 4 * NB], I32, tag=f"boxi{b}")
            nc.sync.dma_start(
                boxi[:, 0 : 4 * NH], hbox[b].rearrange("n f -> (n f)")[None, :]
            )
            nc.sync.dma_start(
                boxi[:, 4 * NH :], obox[b].rearrange("n f -> (n f)")[None, :]
            )
            boxis.append(boxi)
        # features alternate between the two HWDGE rings (doubles the
        # descriptor-generation throughput that was pacing the stream)
        ftg = []  # ftg[b][g]
        for b in range(BPC):
            fts = []
            for g, (c0, c1) in enumerate(GROUPS):
                ft = fpool.tile([128, GMAX, D], BF16, tag="feat")
                eng = nc.sync if g % 2 == 0 else nc.scalar
                eng.dma_start(ft[:, 0 : c1 - c0, :], feats[b, :, c0:c1, :])
                fts.append(ft)
            ftg.append(fts)
        # weights trail the feature stream, split across both rings: they
        # are not needed until ~55us and would otherwise steal DMA-engine
        # time from the latency-critical feature groups
        w1sb = wpool.tile([128, K1, D1], FR, tag="w1")
        nc.sync.dma_start(w1sb[:, 0 : K1 // 2, :], w1[:, 0 : K1 // 2, :])
        nc.scalar.dma_start(w1sb[:, K1 // 2 :, :], w1[:, K1 // 2 :, :])
        w2sb = wpool.tile([128, K2, D2], FR, tag="w2")
        nc.sync.dma_start(w2sb[:], w2[:])
        w3sb = wpool.tile([128, K3, 128], FR, tag="w3")
        nc.scalar.dma_start(w3sb[:], w3[:])

        ident = const.tile([128, 128], F32, tag="ident")
        make_identity(nc, ident[:])
        ones_f32 = const.tile([1, 128], F32, tag="ones_f32")
        nc.vector.memset(ones_f32[:], 1.0)
        ones_row = const.tile([1, 128], FR, tag="ones_row")
        nc.vector.tensor_copy(ones_row[:], ones_f32[:])  # memset can't write f32r

        # ---- box math + field broadcast + first mask piece, per image
        masks = []
        mwrk = const.tile([128, NCHUNK, NB], BF16, tag="mwrk")
        mtmp = const.tile([128, NCHUNK, NB], BF16, tag="mtmp")
        for b in range(BPC):
            masks.append(
                const.tile([128, NCHUNK, NB], BF16, tag=f"mask{b}", name=f"mask{b}")
            )
        bcasts, bcbfs = [], []
        for b in range(BPC):
            boxi = boxis[b]
            boxf = const.tile([1, 5 * NB], FR, tag=f"boxf{b}")
            bfv = boxf[:, 0 : 4 * NB].rearrange("p (f n) -> p f n", n=NB)
            nc.vector.tensor_copy(
                bfv[:, :, 0:NH],
                boxi[:, 0 : 4 * NH].rearrange("p (n f) -> p f n", f=4),
            )
            nc.vector.tensor_copy(
                bfv[:, :, NH:NB],
                boxi[:, 4 * NH :].rearrange("p (n f) -> p f n", f=4),
            )
            dy = spool.tile([1, NB], F32, tag="dy")
            nc.vector.tensor_sub(dy[:], boxf[:, 3 * NB : 4 * NB], boxf[:, NB : 2 * NB])
            dx = spool.tile([1, NB], F32, tag="dx")
            nc.vector.tensor_sub(dx[:], boxf[:, 2 * NB : 3 * NB], boxf[:, 0:NB])
            nc.vector.tensor_mul(dy[:], dy[:], dx[:])
            with nc.allow_low_precision(reason="f32r == f32 bits outside the PE"):
                nc.vector.reciprocal(boxf[:, 4 * NB : 5 * NB], dy[:])

            bps = sm_ps.tile([128, 5 * NB], F32, tag="pp")
            nc.tensor.matmul(bps[:], ones_row[:], boxf[:], start=True, stop=True)
            bcast = const.tile([128, NB], F32, tag=f"inva{b}")  # 1/area
            nc.vector.tensor_copy(bcast[:], bps[:, 4 * NB : 5 * NB])
            bcbf = const.tile([128, 4 * NB], BF16, tag=f"bcbf{b}")  # coords
            nc.vector.tensor_copy(bcbf[:], bps[:, 0 : 4 * NB])
            bcasts.append(bcast)
            bcbfs.append(bcbf)
            for c0, c1 in MASK_PIECES[:1]:
                _mask_piece(nc, masks[b], mwrk, mtmp, ycosb, xcosb, bcbf, c0, c1)
        for c0, c1 in MASK_PIECES[1:]:
            for b in range(BPC):
                _mask_piece(nc, masks[b], mwrk, mtmp, ycosb, xcosb, bcbfs[b], c0, c1)

        # roiT[p, kc, col]: d_in = 128*kc + p; cols = [h(2x8) | o(2x16)]
        roit = const.tile([128, DCH, 2 * NB], FR, tag="roit")
        asb = const.tile([2 * NH, D1], F32, tag="asb")
        bsb = const.tile([2 * NO, D1], F32, tag="bsb")

        # ---- per image: ROI matmuls, transposes, stage-1 half
        for b in range(BPC):
            mask = masks[b]
            pa = roi_ps.tile([NB, 512], F32, tag="roiA")
            pb = roi_ps.tile([NB, 256], F32, tag="roiB")
            for g, (c0, c1) in enumerate(GROUPS):
                ft = ftg[b][g]
                for j in range(c1 - c0):
                    c = c0 + j
                    lhs = mask[:, c, :]
                    nc.tensor.matmul(
                        pa[:], lhs, ft[:, j, 0:512],
                        start=(c == 0), stop=(c == NCHUNK - 1),
                    )
                    mmb = nc.tensor.matmul(
                        pb[:], lhs, ft[:, j, 512:768],
                        start=(c == 0), stop=(c == NCHUNK - 1),
                    )
                    # pa's matmul just loaded the same stationary mask:
                    # skip the redundant LDWEIGHTS for the second half
                    mmb.ins.ldweights = False
            roi = spool.tile([NB, D], F32, tag="roi")
            nc.vector.tensor_copy(roi[:, 0:512], pa[:])
            nc.scalar.copy(roi[:, 512:768], pb[:])

            inva = bcasts[b]
            ptall = sm_ps.tile([128, DCH * NB], F32, tag="pp")
            for t6 in range(DCH):
                nc.tensor.transpose(
                    ptall[:, ts(t6, NB)], roi[:, ts(t6, 128)], ident[:NB, :NB]
                )
            ptv = ptall[:].rearrange("p (t n) -> p t n", n=NB)
            nc.vector.tensor_mul(
                roit[:, :, b * NH : (b + 1) * NH],
                ptv[:, :, 0:NH],
                inva[:, 0:NH].unsqueeze(1).to_broadcast((128, DCH, NH)),
            )
            nc.vector.tensor_mul(
                roit[:, :, 2 * NH + b * NO : 2 * NH + (b + 1) * NO],
                ptv[:, :, NH:NB],
                inva[:, NH:NB].unsqueeze(1).to_broadcast((128, DCH, NO)),
            )


        # ---- stage 1, batched over both images: A = rois_h @ W1h etc.
        A2 = ab_ps.tile([2 * NH, D1], F32, tag="A2")
        B2 = ab_ps.tile([2 * NO, D1], F32, tag="B2")
        for kc in range(DCH):
            nc.tensor.matmul(
                A2[:], roit[:, kc, 0 : 2 * NH], w1sb[:, kc, :],
                start=(kc == 0), stop=(kc == DCH - 1),
            )
            nc.tensor.matmul(
                B2[:], roit[:, kc, 2 * NH :], w1sb[:, DCH + kc, :],
                start=(kc == 0), stop=(kc == DCH - 1),
            )
        nc.vector.tensor_copy(asb[:], A2[:])
        nc.scalar.copy(bsb[:], B2[:])

        # ---- pair expansion + relu -> h1T [128(d1), mc, (img, i, j)]
        h1sb = const.tile([128, MC1, 2 * NPAIR], FR, tag="h1sb")
        for mc in range(MC1):
            atp = sm_ps.tile([128, 2 * NH], F32, tag="pp")
            nc.tensor.transpose(
                atp[:], asb[:, ts(mc, 128)], ident[: 2 * NH, : 2 * NH]
            )
            btp = sm_ps.tile([128, 2 * NO], F32, tag="pp")
            nc.tensor.transpose(
                btp[:], bsb[:, ts(mc, 128)], ident[: 2 * NO, : 2 * NO]
            )
            atsb = spool.tile([128, 2 * NH], F32, tag="atsb")
            nc.scalar.copy(atsb[:], atp[:])
            pre = spool.tile([128, 2 * NPAIR], F32, tag="pre")
            shp3 = (128, NH, NO)
            for b in range(BPC):
                nc.vector.scalar_tensor_tensor(
                    pre[:, b * NPAIR : (b + 1) * NPAIR].rearrange(
                        "p (i j) -> p i j", i=NH
                    ),
                    atsb[:, b * NH : (b + 1) * NH].unsqueeze(2).to_broadcast(shp3),
                    b1sb[:, mc : mc + 1],
                    btp[:, b * NO : (b + 1) * NO].unsqueeze(1).to_broadcast(shp3),
                    mybir.AluOpType.add,
                    mybir.AluOpType.add,
                )
            nc.scalar.activation(
                h1sb[:, mc, :], pre[:], mybir.ActivationFunctionType.Relu
            )

        # ---- stage 2: h2T[m2] = relu(W2[:, m2].T @ h1T + b2)
        h2sb = const.tile([128, MC2, 2 * NPAIR], FR, tag="h2sb")
        for m2 in range(MC2):
            p2 = sm_ps.tile([128, 2 * NPAIR], F32, tag="pp")
            for kc in range(K2):
                nc.tensor.matmul(
                    p2[:], w2sb[:, kc, ts(m2, 128)], h1sb[:, kc, :],
                    start=(kc == 0), stop=(kc == K2 - 1),
                )
            nc.scalar.activation(
                h2sb[:, m2, :], p2[:], mybir.ActivationFunctionType.Relu,
                bias=b2sb[:, m2 : m2 + 1],
            )

        # ---- stage 3: out[img] = h2[img] @ W3 + b3
        osb = const.tile([128, BPC, NREL], F32, tag="osb")
        for b in range(BPC):
            p3 = sm_ps.tile([128, 128], F32, tag="pp")
            for kc in range(K3):
                nc.tensor.matmul(
                    p3[:], h2sb[:, kc, ts(b, 128)], w3sb[:, kc, :],
                    start=(kc == 0), stop=(kc == K3 - 1),
                )
            nc.vector.tensor_add(osb[:, b, :], p3[:, 0:NREL], b3bc[:])
        nc.scalar.dma_start(out[:].rearrange("(i p) n -> p i n", p=128), osb[:])

    nc.compile()
    return nc


def _get_nc():
    if "nc" not in _CACHE:
        _CACHE["nc"] = _build_nc()
    return _CACHE["nc"]


def _in_maps(inputs):
    import ml_dtypes

    feats = np.asarray(inputs["features"], dtype=np.float32).reshape(B, YX, D)
    feats = np.pad(feats, ((0, 0), (0, PADYX - YX), (0, 0)))
    feats = feats.astype(ml_dtypes.bfloat16).reshape(B, 128, NCHUNK, D)
    hb = np.ascontiguousarray(np.asarray(inputs["human_boxes"], dtype=np.int32))
    ob = np.ascontiguousarray(np.asarray(inputs["obj_boxes"], dtype=np.int32))
    yco, xco = _coord_consts()

    def _perm(w):  # [(c p), m] -> [p, c, m] (the SBUF lhsT/rhs chunk layout)
        w = np.asarray(w, dtype=np.float32)
        return np.ascontiguousarray(w.reshape(-1, 128, w.shape[1]).swapaxes(0, 1))

    w3p = np.pad(np.asarray(inputs["W3"], dtype=np.float32), ((0, 0), (0, 128 - NREL)))
    common = {
        "w1": _perm(inputs["W1"]),
        "b1": np.ascontiguousarray(np.asarray(inputs["b1"], dtype=np.float32)),
        "w2": _perm(inputs["W2"]),
        "b2": np.ascontiguousarray(np.asarray(inputs["b2"], dtype=np.float32)),
        "w3": _perm(w3p),
        "b3": np.ascontiguousarray(np.asarray(inputs["b3"], dtype=np.float32)),
        "yco": yco,
        "xco": xco,
    }
    maps = []
    for c in range(NCORES):
        m = dict(common)
        m["feats"] = np.ascontiguousarray(feats[c * BPC : (c + 1) * BPC])
        m["hbox"] = np.ascontiguousarray(hb[c * BPC : (c + 1) * BPC])
        m["obox"] = np.ascontiguousarray(ob[c * BPC : (c + 1) * BPC])
        maps.append(m)
    return maps


def _install_ntff_hook():
    """The agent image's antenv lacks axon_hooks; recreate the NTFF profile
    hook module from the boot shim's ctypes factory so trace=True works."""
    try:
        import antenv.axon_hooks  # noqa: F401

        return
    except ImportError:
        pass
    try:
        from trn_agent_boot.trn_boot import _ntff_profile_via_ctypes
    except ImportError:
        return
    hook = _ntff_profile_via_ctypes("/opt/axon/libaxon_pjrt.so")
    mod = types.ModuleType("antenv.axon_hooks")
    mod.get_axon_ntff_profile_hook = lambda: hook
    mod.set_axon_ntff_profile_hook = lambda h: None
    sys.modules["antenv.axon_hooks"] = mod


def run(trace=False, **inputs):
    if trace:
        _install_ntff_hook()
        # no S3 creds in this container: skip the artifact upload
        from concourse import bass_utils as _bu

        _bu.upload_artifacts = lambda tmpdir: f"file://{tmpdir}"
    nc = _get_nc()
    res = run_bass_kernel_spmd(nc, _in_maps(inputs), list(range(NCORES)), trace=trace)
    out = np.concatenate([res.results[c]["out"] for c in range(NCORES)], axis=0)
    return out.astype(np.float32), res


def kernel(**inputs):
    out, _ = run(trace=False, **inputs)
    return out


# revision 24
# speedup vs baseline: 1.0036x; 1.0036x over previous
"""HOI relation-scoring kernel for Trainium2 (8 NeuronCores, data-parallel).

Full inputs in, full output out. Batch dim (16 images) sharded 2-per-core
across 8 cores; MLP weights replicated.

Per-core dataflow (2 images):
  - Features host-repacked to [128, 43, 768] bf16 per image (flat yx =
    43*p + c, padded 5476 -> 5504): per-partition-contiguous DMA lines,
    no tail chunk; pad rows neutralized via coordinate constants (-1e9).
  - Weights host-permuted to the SBUF chunk layout [p, c, m]: descriptor
    generation is 128 contiguous lines instead of 1536 strided ones
    (the rearranged form cost ~9 us of ACT-sequencer DIRECT2D time and
    stalled everything behind it).
  - Box DMAs ride the SP ring AHEAD of the feature groups; the whole
    box -> mask pipeline runs on the Vector engine in bf16 (2x DVE) and
    is built in two column pieces so the first ROI matmuls start as soon
    as feature group 0 lands. Both images' masks are built before any
    ROI-consuming DVE work so image 1's ROI is never mask-gated.
  - ROI sums: mask chunk [128, 24] bf16 stationary, feature chunk
    [128, 768] bf16 streams -> psum [24, 512] + [24, 256], 43 chunks.
  - MLP in float32r (f32 matmuls lower to 2 half-speed PE instructions;
    f32r does not). Stage 1 is ROI-stationary (lhsT = roiT columns,
    rhs = W1 row-chunk [128, 512]) and runs per image so image 0's half
    fills PE wait gaps during image 1's ROI. Pair expansion
    h1T[d1, (img,i,j)] = A.T + b1 + B.T is a scalar_tensor_tensor per
    d1 chunk on the DVE; stages 2/3 chain in transposed layout
    (weight-stationary, N=256 / N=128 streams, W3 host-padded to 128).
"""

import sys
import types

import numpy as np

for _p in ("/opt/trn_rl_repo",):
    if _p not in sys.path:
        sys.path.insert(0, _p)

from contextlib import ExitStack

from concourse import bacc, mybir, tile
from concourse.bass import ts
from concourse.bass_utils import run_bass_kernel_spmd
from concourse.masks import make_identity

# Problem shapes (hardcoded per contract).
B, H, W, D = 16, 74, 74, 768
NH, NO = 8, 16
NB = NH + NO  # 24 boxes per image
NREL = 117
D1, D2 = 512, 256
NCORES = 8
BPC = B // NCORES  # images per core
YX = H * W  # 5476
NCHUNK = 43
PADYX = 128 * NCHUNK  # 5504
NPAIR = NH * NO  # 128 pairs per image
GROUPS = [(0, 3), (3, 9), (9, 16), (16, 23), (23, 30), (30, 36), (36, 43)]
GMAX = max(c1 - c0 for c0, c1 in GROUPS)
MASK_PIECES = [(0, 3), (3, 43)]

F32 = mybir.dt.float32
FR = mybir.dt.float32r
BF16 = mybir.dt.bfloat16
I32 = mybir.dt.int32

K1 = 2 * D // 128  # 12 chunks of W1 rows (first 6 = human half)
K2 = D1 // 128  # 4
K3 = D2 // 128  # 2
MC1 = D1 // 128  # 4
MC2 = D2 // 128  # 2
DCH = D // 128  # 6

_CACHE = {}


def _coord_consts():
    """Host constants: partition/chunk -> (y, x) coordinate grids [128, 43].

    flat = 43*p + c; invalid (pad) entries get -1e9 so all box compares
    fail and pad rows contribute zero to the ROI sums. bf16 (coords are
    small integers, exact; enables 2x DVE compares).
    """
    import ml_dtypes

    p = np.arange(128)[:, None]
    c = np.arange(NCHUNK)[None, :]
    flat = NCHUNK * p + c
    valid = flat < YX
    y = np.where(valid, flat // W, -1e9).astype(np.float32)
    x = np.where(valid, flat % W, -1e9).astype(np.float32)
    return (
        np.ascontiguousarray(y.astype(ml_dtypes.bfloat16)),
        np.ascontiguousarray(x.astype(ml_dtypes.bfloat16)),
    )


def _mask_piece(nc, mask, mwrk, mtmp, ycosb, xcosb, bcbf, c0, c1):
    """Columns [c0, c1) of one image's indicator mask, all-bf16 on DVE."""
    w = c1 - c0
    shp = (128, w, NB)
    yv = ycosb[:, c0:c1].unsqueeze(2).to_broadcast(shp)
    xv = xcosb[:, c0:c1].unsqueeze(2).to_broadcast(shp)
    x1v = bcbf[:, 0:NB].unsqueeze(1).to_broadcast(shp)
    y1v = bcbf[:, NB : 2 * NB].unsqueeze(1).to_broadcast(shp)
    x2v = bcbf[:, 2 * NB : 3 * NB].unsqueeze(1).to_broadcast(shp)
    y2v = bcbf[:, 3 * NB : 4 * NB].unsqueeze(1).to_broadcast(shp)
    mw = mwrk[:, c0:c1, :]
    mt = mtmp[:, c0:c1, :]
    nc.vector.tensor_tensor(mw, yv, y1v, mybir.AluOpType.is_ge)
    nc.vector.tensor_tensor(mt, yv, y2v, mybir.AluOpType.is_lt)
    nc.vector.tensor_mul(mw, mw, mt)
    nc.vector.tensor_tensor(mt, xv, x1v, mybir.AluOpType.is_ge)
    nc.vector.tensor_mul(mw, mw, mt)
    nc.vector.tensor_tensor(mt, xv, x2v, mybir.AluOpType.is_lt)
    nc.vector.tensor_mul(mask[:, c0:c1, :], mw, mt)


def _build_nc():
    nc = bacc.Bacc("TRN2", target_bir_lowering=False)

    feats = nc.dram_tensor("feats", [BPC, 128, NCHUNK, D], BF16, kind="ExternalInput")
    hbox = nc.dram_tensor("hbox", [BPC, NH, 4], I32, kind="ExternalInput")
    obox = nc.dram_tensor("obox", [BPC, NO, 4], I32, kind="ExternalInput")
    # weights host-permuted to [p, chunk, m]; W3 host-padded to 128 cols
    w1 = nc.dram_tensor("w1", [128, K1, D1], FR, kind="ExternalInput")
    b1 = nc.dram_tensor("b1", [D1], F32, kind="ExternalInput")
    w2 = nc.dram_tensor("w2", [128, K2, D2], FR, kind="ExternalInput")
    b2 = nc.dram_tensor("b2", [D2], F32, kind="ExternalInput")
    w3 = nc.dram_tensor("w3", [128, K3, 128], FR, kind="ExternalInput")
    b3 = nc.dram_tensor("b3", [NREL], F32, kind="ExternalInput")
    yco = nc.dram_tensor("yco", [128, NCHUNK], BF16, kind="ExternalInput")
    xco = nc.dram_tensor("xco", [128, NCHUNK], BF16, kind="ExternalInput")
    out = nc.dram_tensor("out", [BPC * NPAIR, NREL], F32, kind="ExternalOutput")

    with tile.TileContext(nc) as tc, ExitStack() as ctx:
        const = ctx.enter_context(tc.tile_pool(name="const", bufs=1))
        fpool = ctx.enter_context(tc.tile_pool(name="fpool", bufs=11))
        wpool = ctx.enter_context(tc.tile_pool(name="wpool", bufs=1))
        spool = ctx.enter_context(tc.tile_pool(name="spool", bufs=2))
        roi_ps = ctx.enter_context(tc.tile_pool(name="roi_ps", bufs=2, space="PSUM"))
        ab_ps = ctx.enter_context(tc.tile_pool(name="ab_ps", bufs=1, space="PSUM"))
        sm_ps = ctx.enter_context(tc.tile_pool(name="sm_ps", bufs=2, space="PSUM"))

        # ---- ACT ring: small constants go first (cheap descriptor gen)
        ycosb = const.tile([128, NCHUNK], BF16, tag="ycosb")
        nc.scalar.dma_start(ycosb[:], yco[:])
        xcosb = const.tile([128, NCHUNK], BF16, tag="xcosb")
        nc.scalar.dma_start(xcosb[:], xco[:])
        b1sb = const.tile([128, MC1], F32, tag="b1")
        nc.scalar.dma_start(b1sb[:], b1[:].rearrange("(c p) -> p c", p=128))
        b2sb = const.tile([128, MC2], F32, tag="b2")
        nc.scalar.dma_start(b2sb[:], b2[:].rearrange("(c p) -> p c", p=128))
        b3bc = const.tile([128, NREL], F32, tag="b3bc")
        nc.scalar.dma_start(b3bc[:], b3[None, :].to_broadcast((128, NREL)))

        # ---- SP ring: tiny box DMAs first, then the feature stream
        boxis = []
        for b in range(BPC):
            boxi = const.tile([1, 4 * NB], I32, tag=f"boxi{b}")
            nc.sync.dma_start(
                boxi[:, 0 : 4 * NH], hbox[b].rearrange("n f -> (n f)")[None, :]
            )
            nc.sync.dma_start(
                boxi[:, 4 * NH :], obox[b].rearrange("n f -> (n f)")[None, :]
            )
            boxis.append(boxi)
        # features alternate between the two HWDGE rings (doubles the
        # descriptor-generation throughput that was pacing the stream)
        ftg = []  # ftg[b][g]
        for b in range(BPC):
            fts = []
            for g, (c0, c1) in enumerate(GROUPS):
                ft = fpool.tile([128, GMAX, D], BF16, tag="feat")
                eng = nc.sync if g % 2 == 0 else nc.scalar
                eng.dma_start(ft[:, 0 : c1 - c0, :], feats[b, :, c0:c1, :])
                fts.append(ft)
            ftg.append(fts)
        # weights trail the feature stream, split across both rings: they
        # are not needed until ~55us and would otherwise steal DMA-engine
        # time from the latency-critical feature groups
        w1sb = wpool.tile([128, K1, D1], FR, tag="w1")
        nc.sync.dma_start(w1sb[:, 0 : K1 // 2, :], w1[:, 0 : K1 // 2, :])
        nc.scalar.dma_start(w1sb[:, K1 // 2 :, :], w1[:, K1 // 2 :, :])
        w2sb = wpool.tile([128, K2, D2], FR, tag="w2")
        nc.sync.dma_start(w2sb[:], w2[:])
        w3sb = wpool.tile([128, K3, 128], FR, tag="w3")
        nc.scalar.dma_start(w3sb[:], w3[:])

        ident = const.tile([128, 128], F32, tag="ident")
        make_identity(nc, ident[:])
        ones_f32 = const.tile([1, 128], F32, tag="ones_f32")
        nc.vector.memset(ones_f32[:], 1.0)
        ones_row = const.tile([1, 128], FR, tag="ones_row")
        nc.vector.tensor_copy(ones_row[:], ones_f32[:])  # memset can't write f32r

        # ---- box math + field broadcast + first mask piece, per image
        masks = []
        mwrk = const.tile([128, NCHUNK, NB], BF16, tag="mwrk")
        mtmp = const.tile([128, NCHUNK, NB], BF16, tag="mtmp")
        for b in range(BPC):
            masks.append(
                const.tile([128, NCHUNK, NB], BF16, tag=f"mask{b}", name=f"mask{b}")
            )
        bcasts, bcbfs = [], []
        for b in range(BPC):
            boxi = boxis[b]
            boxf = const.tile([1, 5 * NB], FR, tag=f"boxf{b}")
            bfv = boxf[:, 0 : 4 * NB].rearrange("p (f n) -> p f n", n=NB)
            nc.vector.tensor_copy(
                bfv[:, :, 0:NH],
                boxi[:, 0 : 4 * NH].rearrange("p (n f) -> p f n", f=4),
            )
            nc.vector.tensor_copy(
                bfv[:, :, NH:NB],
                boxi[:, 4 * NH :].rearrange("p (n f) -> p f n", f=4),
            )
            dy = spool.tile([1, NB], F32, tag="dy")
            nc.vector.tensor_sub(dy[:], boxf[:, 3 * NB : 4 * NB], boxf[:, NB : 2 * NB])
            dx = spool.tile([1, NB], F32, tag="dx")
            nc.vector.tensor_sub(dx[:], boxf[:, 2 * NB : 3 * NB], boxf[:, 0:NB])
            nc.vector.tensor_mul(dy[:], dy[:], dx[:])
            with nc.allow_low_precision(reason="f32r == f32 bits outside the PE"):
                nc.vector.reciprocal(boxf[:, 4 * NB : 5 * NB], dy[:])

            bps = sm_ps.tile([128, 5 * NB], F32, tag="pp")
            nc.tensor.matmul(bps[:], ones_row[:], boxf[:], start=True, stop=True)
            bcast = const.tile([128, NB], F32, tag=f"inva{b}")  # 1/area
            nc.vector.tensor_copy(bcast[:], bps[:, 4 * NB : 5 * NB])
            bcbf = const.tile([128, 4 * NB], BF16, tag=f"bcbf{b}")  # coords
            nc.vector.tensor_copy(bcbf[:], bps[:, 0 : 4 * NB])
            bcasts.append(bcast)
            bcbfs.append(bcbf)
            for c0, c1 in MASK_PIECES[:1]:
                _mask_piece(nc, masks[b], mwrk, mtmp, ycosb, xcosb, bcbf, c0, c1)
        for c0, c1 in MASK_PIECES[1:]:
            for b in range(BPC):
                _mask_piece(nc, masks[b], mwrk, mtmp, ycosb, xcosb, bcbfs[b], c0, c1)

        # roiT[p, kc, col]: d_in = 128*kc + p; cols = [h(2x8) | o(2x16)]
        roit = const.tile([128, DCH, 2 * NB], FR, tag="roit")
        asb = const.tile([2 * NH, D1], F32, tag="asb")
        bsb = const.tile([2 * NO, D1], F32, tag="bsb")

        # ---- per image: ROI matmuls, transposes, stage-1 half
        for b in range(BPC):
            mask = masks[b]
            pa = roi_ps.tile([NB, 512], F32, tag="roiA")
            pb = roi_ps.tile([NB, 256], F32, tag="roiB")
            for g, (c0, c1) in enumerate(GROUPS):
                ft = ftg[b][g]
                for j in range(c1 - c0):
                    c = c0 + j
                    lhs = mask[:, c, :]
                    nc.tensor.matmul(
                        pa[:], lhs, ft[:, j, 0:512],
                        start=(c == 0), stop=(c == NCHUNK - 1),
                    )
                    nc.tensor.matmul(
                        pb[:], lhs, ft[:, j, 512:768],
                        start=(c == 0), stop=(c == NCHUNK - 1),
                    )
            roi = spool.tile([NB, D], F32, tag="roi")
            nc.vector.tensor_copy(roi[:, 0:512], pa[:])
            nc.scalar.copy(roi[:, 512:768], pb[:])

            inva = bcasts[b]
            ptall = sm_ps.tile([128, DCH * NB], F32, tag="pp")
            for t6 in range(DCH):
                nc.tensor.transpose(
                    ptall[:, ts(t6, NB)], roi[:, ts(t6, 128)], ident[:NB, :NB]
                )
            ptv = ptall[:].rearrange("p (t n) -> p t n", n=NB)
            nc.vector.tensor_mul(
                roit[:, :, b * NH : (b + 1) * NH],
                ptv[:, :, 0:NH],
                inva[:, 0:NH].unsqueeze(1).to_broadcast((128, DCH, NH)),
            )
            nc.vector.tensor_mul(
                roit[:, :, 2 * NH + b * NO : 2 * NH + (b + 1) * NO],
                ptv[:, :, NH:NB],
                inva[:, NH:NB].unsqueeze(1).to_broadcast((128, DCH, NO)),
            )


        # ---- stage 1, batched over both images: A = rois_h @ W1h etc.
        A2 = ab_ps.tile([2 * NH, D1], F32, tag="A2")
        B2 = ab_ps.tile([2 * NO, D1], F32, tag="B2")
        for kc in range(DCH):
            nc.tensor.matmul(
                A2[:], roit[:, kc, 0 : 2 * NH], w1sb[:, kc, :],
                start=(kc == 0), stop=(kc == DCH - 1),
            )
            nc.tensor.matmul(
                B2[:], roit[:, kc, 2 * NH :], w1sb[:, DCH + kc, :],
                start=(kc == 0), stop=(kc == DCH - 1),
            )
        nc.vector.tensor_copy(asb[:], A2[:])
        nc.scalar.copy(bsb[:], B2[:])

        # ---- pair expansion + relu -> h1T [128(d1), mc, (img, i, j)]
        h1sb = const.tile([128, MC1, 2 * NPAIR], FR, tag="h1sb")
        for mc in range(MC1):
            atp = sm_ps.tile([128, 2 * NH], F32, tag="pp")
            nc.tensor.transpose(
                atp[:], asb[:, ts(mc, 128)], ident[: 2 * NH, : 2 * NH]
            )
            btp = sm_ps.tile([128, 2 * NO], F32, tag="pp")
            nc.tensor.transpose(
                btp[:], bsb[:, ts(mc, 128)], ident[: 2 * NO, : 2 * NO]
            )
            atsb = spool.tile([128, 2 * NH], F32, tag="atsb")
            nc.scalar.copy(atsb[:], atp[:])
            pre = spool.tile([128, 2 * NPAIR], F32, tag="pre")
            shp3 = (128, NH, NO)
            for b in range(BPC):
                nc.vector.scalar_tensor_tensor(
                    pre[:, b * NPAIR : (b + 1) * NPAIR].rearrange(
                        "p (i j) -> p i j", i=NH
                    ),
                    atsb[:, b * NH : (b + 1) * NH].unsqueeze(2).to_broadcast(shp3),
                    b1sb[:, mc : mc + 1],
                    btp[:, b * NO : (b + 1) * NO].unsqueeze(1).to_broadcast(shp3),
                    mybir.AluOpType.add,
                    mybir.AluOpType.add,
                )
            nc.scalar.activation(
                h1sb[:, mc, :], pre[:], mybir.ActivationFunctionType.Relu
            )

        # ---- stage 2: h2T[m2] = relu(W2[:, m2].T @ h1T + b2)
        h2sb = const.tile([128, MC2, 2 * NPAIR], FR, tag="h2sb")
        for m2 in range(MC2):
            p2 = sm_ps.tile([128, 2 * NPAIR], F32, tag="pp")
            for kc in range(K2):
                nc.tensor.matmul(
                    p2[:], w2sb[:, kc, ts(m2, 128)], h1sb[:, kc, :],
                    start=(kc == 0), stop=(kc == K2 - 1),
                )
            nc.scalar.activation(
                h2sb[:, m2, :], p2[:], mybir.ActivationFunctionType.Relu,
                bias=b2sb[:, m2 : m2 + 1],
            )

        # ---- stage 3: out[img] = h2[img] @ W3 + b3
        osb = const.tile([128, BPC, NREL], F32, tag="osb")
        for b in range(BPC):
            p3 = sm_ps.tile([128, 128], F32, tag="pp")
            for kc in range(K3):
                nc.tensor.matmul(
                    p3[:], h2sb[:, kc, ts(b, 128)], w3sb[:, kc, :],
                    start=(kc == 0), stop=(kc == K3 - 1),
                )
            nc.vector.tensor_add(osb[:, b, :], p3[:, 0:NREL], b3bc[:])
        nc.scalar.dma_start(out[:].rearrange("(i p) n -> p i n", p=128), osb[:])

    nc.compile()
    return nc


def _get_nc():
    if "nc" not in _CACHE:
        _CACHE["nc"] = _build_nc()
    return _CACHE["nc"]


def _in_maps(inputs):
    import ml_dtypes

    feats = np.asarray(inputs["features"], dtype=np.float32).reshape(B, YX, D)
    feats = np.pad(feats, ((0, 0), (0, PADYX - YX), (0, 0)))
    feats = feats.astype(ml_dtypes.bfloat16).reshape(B, 128, NCHUNK, D)
    hb = np.ascontiguousarray(np.asarray(inputs["human_boxes"], dtype=np.int32))
    ob = np.ascontiguousarray(np.asarray(inputs["obj_boxes"], dtype=np.int32))
    yco, xco = _coord_consts()

    def _perm(w):  # [(c p), m] -> [p, c, m] (the SBUF lhsT/rhs chunk layout)
        w = np.asarray(w, dtype=np.float32)
        return np.ascontiguousarray(w.reshape(-1, 128, w.shape[1]).swapaxes(0, 1))

    w3p = np.pad(np.asarray(inputs["W3"], dtype=np.float32), ((0, 0), (0, 128 - NREL)))
    common = {
        "w1": _perm(inputs["W1"]),
        "b1": np.ascontiguousarray(np.asarray(inputs["b1"], dtype=np.float32)),
        "w2": _perm(inputs["W2"]),
        "b2": np.ascontiguousarray(np.asarray(inputs["b2"], dtype=np.float32)),
        "w3": _perm(w3p),
        "b3": np.ascontiguousarray(np.asarray(inputs["b3"], dtype=np.float32)),
        "yco": yco,
        "xco": xco,
    }
    maps = []
    for c in range(NCORES):
        m = dict(common)
        m["feats"] = np.ascontiguousarray(feats[c * BPC : (c + 1) * BPC])
        m["hbox"] = np.ascontiguousarray(hb[c * BPC : (c + 1) * BPC])
        m["obox"] = np.ascontiguousarray(ob[c * BPC : (c + 1) * BPC])
        maps.append(m)
    return maps


def _install_ntff_hook():
    """The agent image's antenv lacks axon_hooks; recreate the NTFF profile
    hook module from the boot shim's ctypes factory so trace=True works."""
    try:
        import antenv.axon_hooks  # noqa: F401

        return
    except ImportError:
        pass
    try:
        from trn_agent_boot.trn_boot import _ntff_profile_via_ctypes
    except ImportError:
        return
    hook = _ntff_profile_via_ctypes("/opt/axon/libaxon_pjrt.so")
    mod = types.ModuleType("antenv.axon_hooks")
    mod.get_axon_ntff_profile_hook = lambda: hook
    mod.set_axon_ntff_profile_hook = lambda h: None
    sys.modules["antenv.axon_hooks"] = mod


def run(trace=False, **inputs):
    if trace:
        _install_ntff_hook()
        # no S3 creds in this container: skip the artifact upload
        from concourse import bass_utils as _bu

        _bu.upload_artifacts = lambda tmpdir: f"file://{tmpdir}"
    nc = _get_nc()
    res = run_bass_kernel_spmd(nc, _in_maps(inputs), list(range(NCORES)), trace=trace)
    out = np.concatenate([res.results[c]["out"] for c in range(NCORES)], axis=0)
    return out.astype(np.float32), res


def kernel(**inputs):
    out, _ = run(trace=False, **inputs)
    return out
